# revision 3
# baseline (speedup 1.0000x reference)
# Trainium2 Bass kernel for nn_Lowrank_Spattention (sparse_attention).
#
# Reference math (per batch b, n=8192 tokens, f=256 features, h=4 heads,
# r=64 latent ranks, d=64 head dim):
#   q    = z @ Wq + bq                    (n, h*d)
#   attn = einsum(q, K)/sqrt(d)          (n, h*r)   == z @ M + ab
#            where M[:, h*r+j] = (Wq_h @ K_h^T)/8,  ab = bq @ K^T/8
#   xv   = x @ Wv + bv                    (n, h*d)
#   pooled = softmax_r(attn)^T-pool of xv (r, h*d)
#   v    = softmax_n(attn) @ pooled       (n, h*d)
#   out  = sig(alpha)*xv + sig(beta)*v
#
# Kernel strategy (one NeuronCore per batch element, 8 cores, no collectives):
#   E = exp(attn) (no max-subtraction needed; attn ~ N(0,1)), stored bf16.
#   rowsum_h[n] = sum_r E; Eh = E / rowsum (the row-softmax numerator).
#   Pass A (per 128-row chunk): z^T via PE transpose -> attn matmul ->
#     exp -> rowsum/recip -> Eh -> accumulate G = Eh^T @ [x | 1 | rowsums]
#     into PSUM across all 8192 rows.  G gives pooled (vs x), esum (for bv)
#     and colsum (softmax_n denominator) in one matmul stream.
#   Finalize (tiny): pooled = G[:, :256] @ Wv + esum*bv;
#     PS = sig(beta) * pooled / colsum, laid out block-diagonal (bf16).
#   Pass B (per chunk): out = x @ (sig(alpha)Wv) + E @ PS_bd + bias, with
#     x^T/E^T via PE transposes, all accumulated in one PSUM group.
#
# The v-path (everything through E) is computed in bf16: its contribution
# to the output is scaled by sig(beta)=0.01 and pooled averages 8192 rows,
# so its relative error is damped ~1e2-1e4x.  The xv-path uses float32r.

import math

import numpy as np

import concourse.bass as bass
import concourse.mybir as mybir
import concourse.tile as tile
from concourse import bacc
from concourse.masks import make_identity

B, N, DIM = 8, 8192, 256
HEAD, RANK, HDIM = 4, 64, 64
NCORES = 8
CHUNK = 128                 # rows per compute chunk
NCHUNK = N // CHUNK         # 64
SUPER = 8                   # chunks per DMA super-chunk (1 MiB)
NSUPER = NCHUNK // SUPER    # 8

F32 = mybir.dt.float32
F32R = mybir.dt.float32r
BF16 = mybir.dt.bfloat16


def _r(ap, dt):
    return ap.bitcast(dt)


def build_body(tc, outs, ins):
    """Emit the per-core program.  outs/ins are dicts of bass.APs."""
    nc = tc.nc
    z, x = ins["z"], ins["x"]
    out = outs["out"]
    has_ab = ins.get("ab_row") is not None

    with (
        tc.tile_pool(name="consts", bufs=1) as consts,
        tc.tile_pool(name="resident", bufs=1) as resident,
    ):
        # ---- constants ----
        ident_f = consts.tile([128, 128], F32R)
        nc.gpsimd.memset(ident_f.bitcast(F32), 0.0)
        nc.gpsimd.affine_select(
            out=ident_f, in_=ident_f,
            compare_op=mybir.AluOpType.not_equal, fill=1.0,
            base=0, pattern=[[-1, 128]], channel_multiplier=1,
        )
        ident_bf = consts.tile([128, 128], BF16)
        make_identity(nc, ident_bf)

        mq_s = consts.tile([128, 2, DIM], BF16)
        nc.sync.dma_start(out=mq_s, in_=ins["mq"].rearrange("(t p) n -> p t n", p=128))
        swv_s = consts.tile([128, 2, DIM], F32R)
        nc.sync.dma_start(out=swv_s, in_=ins["swv"].rearrange("(t p) n -> p t n", p=128))
        wv_s = consts.tile([128, 2, DIM], F32R)
        nc.sync.dma_start(out=wv_s, in_=ins["wv"].rearrange("(t p) n -> p t n", p=128))
        # broadcast rows across partitions (SWDGE replication)
        bvp_bc = consts.tile([128, DIM], F32)
        nc.gpsimd.dma_start(out=bvp_bc, in_=ins["bv_row"].to_broadcast([128, DIM]))
        biasout_bc = consts.tile([128, DIM], F32)
        nc.gpsimd.dma_start(
            out=biasout_bc, in_=ins["biasout_row"].to_broadcast([128, DIM])
        )
        sbcol_s = consts.tile([128, 2], F32)
        nc.sync.dma_start(out=sbcol_s, in_=ins["sbcol"])
        if has_ab:
            ones_row = consts.tile([1, 128], BF16)
            nc.vector.memset(ones_row, 1.0)
            ab_s = consts.tile([1, DIM], BF16)
            nc.sync.dma_start(out=ab_s, in_=ins["ab_row"])

        # ---- residents ----
        x_res = resident.tile([128, NCHUNK, DIM], F32R)
        e_all = resident.tile([128, NCHUNK, DIM], BF16)
        psbd = resident.tile([128, 2, DIM], BF16)   # block-diag PS (stage-3 rhs)

        with (
            tc.tile_pool(name="pa_sbuf", bufs=2) as pa,
            tc.tile_pool(name="pa_psum", bufs=2, space="PSUM") as pap,
            tc.tile_pool(name="g_psum", bufs=1, space="PSUM") as gp,
            tc.tile_pool(name="fin_sbuf", bufs=1) as fin,
            tc.tile_pool(name="fin_psum", bufs=1, space="PSUM") as finp,
        ):
            g0 = gp.tile([128, 262], F32, tag="g0")
            g1 = gp.tile([128, 262], F32, tag="g1")

            # ================= Pass A =================
            for sc in range(NSUPER):
                zstage = pa.tile([128, SUPER, DIM], F32R, tag="zstage")
                nc.sync.dma_start(
                    out=zstage,
                    in_=z[sc * SUPER * CHUNK : (sc + 1) * SUPER * CHUNK, :].rearrange(
                        "(c p) f -> p c f", p=128
                    ),
                )
                nc.sync.dma_start(
                    out=x_res[:, sc * SUPER : (sc + 1) * SUPER, :],
                    in_=x[sc * SUPER * CHUNK : (sc + 1) * SUPER * CHUNK, :].rearrange(
                        "(c p) f -> p c f", p=128
                    ),
                )
                for cc in range(SUPER):
                    c = sc * SUPER + cc
                    z_c = zstage[:, cc, :]
                    # z^T (PE transpose, f32r) -> PSUM -> bf16 SBUF via ACT
                    zt_ps = pap.tile([128, 2, 128], F32R, tag="zt_ps")
                    for kt in range(2):
                        nc.tensor.transpose(
                            zt_ps[:, kt, :],
                            z_c[:, kt * 128 : (kt + 1) * 128],
                            ident_f,
                        )
                    zt = pa.tile([128, 2, 128], BF16, tag="zt")
                    nc.scalar.copy(zt, zt_ps)
                    # attn = z @ M (+ ab)
                    attn_ps = pap.tile([128, DIM], F32, tag="attn_ps")
                    nc.tensor.matmul(
                        attn_ps, zt[:, 0, :], mq_s[:, 0, :], start=True, stop=False
                    )
                    nc.tensor.matmul(
                        attn_ps, zt[:, 1, :], mq_s[:, 1, :],
                        start=False, stop=not has_ab,
                    )
                    if has_ab:
                        nc.tensor.matmul(
                            attn_ps, ones_row, ab_s, start=False, stop=True
                        )
                    # E = exp(attn)  (bf16, resident)
                    nc.scalar.activation(
                        e_all[:, c, :], attn_ps, mybir.ActivationFunctionType.Exp
                    )
                    # per-head rowsum -> aux cols 1..4; aux col 0 = 1.0
                    aux = pa.tile([128, 6], F32R, tag="aux")
                    nc.gpsimd.memset(aux[:, 0:6:5].bitcast(F32), 1.0)
                    with nc.allow_low_precision(reason="rowsum feeds damped v-path"):
                        nc.vector.tensor_reduce(
                            aux[:, 1:5],
                            e_all[:, c, :].rearrange("p (h r) -> p h r", h=HEAD),
                            axis=mybir.AxisListType.X,
                            op=mybir.AluOpType.add,
                        )
                    rcp = pa.tile([128, 4], F32, tag="rcp")
                    nc.vector.reciprocal(rcp, aux[:, 1:5])
                    # Eh = E / rowsum (f32)
                    eh = pa.tile([128, HEAD, RANK], F32R, tag="eh")
                    rcp_bc = bass.AP(
                        tensor=rcp.tensor,
                        offset=rcp.offset,
                        ap=[rcp.ap[0], [1, 4], [0, RANK]],
                    )
                    nc.gpsimd.tensor_tensor(
                        out=eh,
                        in0=e_all[:, c, :].rearrange("p (h r) -> p h r", h=HEAD),
                        in1=rcp_bc,
                        op=mybir.AluOpType.mult,
                    )
                    # G += Eh^T @ [x | aux]
                    ehf = eh.rearrange("p h r -> p (h r)")
                    for gi, g in enumerate((g0, g1)):
                        lhs = ehf[:, gi * 128 : (gi + 1) * 128]
                        nc.tensor.matmul(
                            g[:, 0:256], lhs, x_res[:, c, :],
                            start=(c == 0), stop=False,
                        )
                        nc.tensor.matmul(
                            g[:, 256:262], lhs, aux,
                            start=False, stop=(c == NCHUNK - 1),
                        )

            # ================= Finalize =================
            for gi, g in enumerate((g0, g1)):
                gs = fin.tile([128, 262], F32R, tag=f"gs{gi}")
                nc.vector.tensor_copy(gs, g)
                gt_ps = finp.tile([128, 2, 128], F32R, tag="gt_ps")
                for kt in range(2):
                    nc.tensor.transpose(
                        gt_ps[:, kt, :],
                        gs[:, kt * 128 : (kt + 1) * 128],
                        ident_f,
                    )
                gt = fin.tile([128, 2, 128], F32R, tag="gt")
                nc.scalar.copy(gt, gt_ps)
                p_ps = finp.tile([128, 128], F32, tag="p_ps")
                for kt in range(2):
                    nc.tensor.matmul(
                        p_ps,
                        gt[:, kt, :],
                        wv_s[:, kt, gi * 128 : (gi + 1) * 128],
                        start=(kt == 0), stop=(kt == 1),
                    )
                # pooled = p_ps + esum * bv
                pool_s = fin.tile([128, 128], F32, tag=f"pool_s{gi}")
                nc.vector.scalar_tensor_tensor(
                    out=pool_s,
                    in0=bvp_bc[:, gi * 128 : (gi + 1) * 128],
                    scalar=gs[:, 256:257],
                    in1=p_ps,
                    op0=mybir.AluOpType.mult,
                    op1=mybir.AluOpType.add,
                )
                # colsum (col 257 for even head rows, 258 for odd head rows)
                cs = fin.tile([128, 1], F32, tag=f"cs{gi}")
                h0, h1 = 2 * gi, 2 * gi + 1
                nc.vector.tensor_copy(cs[0:64, :], gs[0:64, 257 + h0 : 258 + h0])
                nc.vector.tensor_copy(cs[64:128, :], gs[64:128, 257 + h1 : 258 + h1])
                rcs = fin.tile([128, 1], F32, tag=f"rcs{gi}")
                nc.vector.reciprocal(rcs, cs)
                nc.vector.tensor_mul(rcs, rcs, sbcol_s[:, gi : gi + 1])
                # PS block-diag (bf16): rows = this pair's (h even r | h odd r)
                if gi == 0:
                    nc.gpsimd.memset(psbd, 0.0)
                nc.vector.tensor_scalar_mul(
                    psbd[0:64, gi, gi * 128 : gi * 128 + 64],
                    pool_s[0:64, 0:64],
                    rcs[0:64, :],
                )
                nc.vector.tensor_scalar_mul(
                    psbd[64:128, gi, gi * 128 + 64 : gi * 128 + 128],
                    pool_s[64:128, 64:128],
                    rcs[64:128, :],
                )

        # ================= Pass B =================
        with (
            tc.tile_pool(name="pb_sbuf", bufs=2) as pb,
            tc.tile_pool(name="pb_psum", bufs=2, space="PSUM") as pbp,
        ):
            for sc in range(NSUPER):
                ostage = pb.tile([128, SUPER, DIM], F32, tag="ostage")
                for cc in range(SUPER):
                    c = sc * SUPER + cc
                    xt_ps = pbp.tile([128, 2, 128], F32R, tag="xt_ps")
                    for kt in range(2):
                        nc.tensor.transpose(
                            xt_ps[:, kt, :],
                            x_res[:, c, kt * 128 : (kt + 1) * 128],
                            ident_f,
                        )
                    xt = pb.tile([128, 2, 128], F32R, tag="xt")
                    nc.scalar.copy(xt, xt_ps)
                    et_ps = pbp.tile([128, 2, 128], BF16, tag="et_ps")
                    for kt in range(2):
                        nc.tensor.transpose(
                            et_ps[:, kt, :],
                            e_all[:, c, kt * 128 : (kt + 1) * 128],
                            ident_bf,
                        )
                    et = pb.tile([128, 2, 128], BF16, tag="et")
                    nc.scalar.copy(et, et_ps)
                    out_ps = pbp.tile([128, DIM], F32, tag="out_ps")
                    nc.tensor.matmul(
                        out_ps, xt[:, 0, :], swv_s[:, 0, :],
                        start=True, stop=False,
                    )
                    nc.tensor.matmul(
                        out_ps, xt[:, 1, :], swv_s[:, 1, :],
                        start=False, stop=False,
                    )
                    nc.tensor.matmul(
                        out_ps, et[:, 0, :], psbd[:, 0, :], start=False, stop=False
                    )
                    nc.tensor.matmul(
                        out_ps, et[:, 1, :], psbd[:, 1, :], start=False, stop=True
                    )
                    nc.vector.tensor_add(ostage[:, cc, :], out_ps, biasout_bc)
                nc.sync.dma_start(
                    out=out[sc * SUPER * CHUNK : (sc + 1) * SUPER * CHUNK, :].rearrange(
                        "(c p) f -> p c f", p=128
                    ),
                    in_=ostage,
                )


def fold_params(Wq, bq, K, Wv, bv, alpha, beta):
    """Host-side folding of the tiny parameter tensors (all O(256^2))."""
    Wq = np.asarray(Wq, np.float64)
    bq = np.asarray(bq, np.float64)
    K = np.asarray(K, np.float64)
    Wv = np.asarray(Wv, np.float64)
    bv = np.asarray(bv, np.float64)
    sa = 1.0 / (1.0 + np.exp(-np.asarray(alpha, np.float64)[:, 0]))  # (HEAD,)
    sb = 1.0 / (1.0 + np.exp(-np.asarray(beta, np.float64)[:, 0]))
    scale = 1.0 / math.sqrt(HDIM)
    # M[:, h*RANK + r] = Wq_h @ K_h^T / sqrt(d)
    M = np.zeros((DIM, HEAD * RANK))
    ab = np.zeros((HEAD * RANK,))
    for h in range(HEAD):
        Kh = K[:, h, :]  # (RANK, HDIM)
        M[:, h * RANK : (h + 1) * RANK] = Wq[:, h * HDIM : (h + 1) * HDIM] @ Kh.T * scale
        ab[h * RANK : (h + 1) * RANK] = (bq[h * HDIM : (h + 1) * HDIM] @ Kh.T) * scale
    sa_vec = np.repeat(sa, HDIM)  # (256,)
    swv = Wv * sa_vec[None, :]
    biasout = bv * sa_vec
    sbcol = np.zeros((128, 2))
    for gi in range(2):
        sbcol[0:64, gi] = sb[2 * gi]
        sbcol[64:128, gi] = sb[2 * gi + 1]
    return {
        "mq": M.astype(np.float32),
        "ab": ab.astype(np.float32),
        "swv": swv.astype(np.float32),
        "wv": Wv.astype(np.float32),
        "bv_row": bv.astype(np.float32).reshape(1, DIM),
        "biasout_row": biasout.astype(np.float32).reshape(1, DIM),
        "sbcol": sbcol.astype(np.float32),
    }


def build_nc(has_ab):
    nc = bacc.Bacc("TRN2", target_bir_lowering=False, debug=False,
                   enable_asserts=False)
    ins = {
        "z": nc.dram_tensor("z", [N, DIM], F32R, kind="ExternalInput").ap(),
        "x": nc.dram_tensor("x", [N, DIM], F32R, kind="ExternalInput").ap(),
        "mq": nc.dram_tensor("mq", [DIM, DIM], BF16, kind="ExternalInput").ap(),
        "swv": nc.dram_tensor("swv", [DIM, DIM], F32R, kind="ExternalInput").ap(),
        "wv": nc.dram_tensor("wv", [DIM, DIM], F32R, kind="ExternalInput").ap(),
        "bv_row": nc.dram_tensor("bv_row", [1, DIM], F32, kind="ExternalInput").ap(),
        "biasout_row": nc.dram_tensor(
            "biasout_row", [1, DIM], F32, kind="ExternalInput"
        ).ap(),
        "sbcol": nc.dram_tensor("sbcol", [128, 2], F32, kind="ExternalInput").ap(),
        "ab_row": (
            nc.dram_tensor("ab_row", [1, DIM], BF16, kind="ExternalInput").ap()
            if has_ab
            else None
        ),
    }
    outs = {"out": nc.dram_tensor("out", [N, DIM], F32, kind="ExternalOutput").ap()}
    with tile.TileContext(nc) as tc:
        build_body(tc, outs, ins)
    nc.compile()
    return nc


LAST_RESULTS = None


def kernel(x, z, Wq, bq, K, Wv, bv, alpha, beta):
    global LAST_RESULTS
    from concourse.bass_utils import run_bass_kernel_spmd

    x = np.ascontiguousarray(np.asarray(x, np.float32))
    z = np.ascontiguousarray(np.asarray(z, np.float32))
    p = fold_params(Wq, bq, K, Wv, bv, alpha, beta)
    has_ab = bool(np.any(p["ab"] != 0.0))

    nc = build_nc(has_ab)

    common = {
        "mq": p["mq"].astype(np.dtype("bfloat16") if False else np.float32),
        "swv": p["swv"],
        "wv": p["wv"],
        "bv_row": p["bv_row"],
        "biasout_row": p["biasout_row"],
        "sbcol": p["sbcol"],
    }
    # bf16 tensors need ml_dtypes bfloat16 numpy arrays
    import ml_dtypes

    common["mq"] = p["mq"].astype(ml_dtypes.bfloat16)
    if has_ab:
        common["ab_row"] = p["ab"].reshape(1, DIM).astype(ml_dtypes.bfloat16)

    in_maps = [dict(common, z=z[i], x=x[i]) for i in range(NCORES)]
    res = run_bass_kernel_spmd(nc, in_maps, core_ids=list(range(NCORES)))
    LAST_RESULTS = res
    out = np.stack([res.results[i]["out"] for i in range(NCORES)], axis=0)
    return out.astype(np.float32)
